# revision 8
# baseline (speedup 1.0000x reference)
"""GQA attention kernel for Trainium2, 8 NeuronCores.

Sharding: 8-way tensor parallel over query heads; every core processes BOTH
batches. Core c owns kv group g = c//2 (its K/V projection) and query heads
{2c, 2c+1} (columns c*256:(c+1)*256 of Wq, rows c*256:(c+1)*256 of Wo).

Host <-> device I/O is minimized (the axon tunnel runs at ~30-60 MB/s, which
dominates wall time; device compute is ~1 ms):
  - x is shipped bf16 (int8 was tried: its quantization noise amplifies
    ~1.7x through the attention scores and blew the error budget), 1/8th of
    the token rows per core ([512, 2048], 2 MB), AllGather'd on device.
  - Each core computes its 2-head partial y for all 4096 token rows in f32,
    an 8-way ReduceScatter(add) leaves each core with 512 final rows, which
    are quantized on device to int8 with per-row f32 scales -> each core
    outputs ~1 MB. Host dequantizes and adds bo.
  - Masks/identity/ones are inline Const tensors baked into the NEFF.
  - Weights are uploaded bf16 once and cached on device across calls (content
    fingerprint guard); x upload + execute + y download happen every call.

Device compute (per core): x transposed to [d, t] tiles via PE transposes, then the baseline's proven pipeline: QT/KT/V projections,
causal attention with transposed scores (ST [s, t]), softmax denominators via
ones-vector matmuls, normalization deferred to attnT evacuation, diagonal
score blocks masked multiplicatively post-exp, O-projection from attnT
without transposes. Compute dtype bf16, accumulation f32.
"""

import sys

sys.path.insert(0, "/opt/trn_rl_repo")

import numpy as np
import ml_dtypes

import concourse.bass as bass
import concourse.tile as tile
from concourse import mybir
from concourse import bass2jax
from concourse.bass_utils import run_bass_kernel_spmd  # noqa: F401 (fallback path)

BF = mybir.dt.bfloat16
F32 = mybir.dt.float32
I8 = mybir.dt.int8
NPBF = ml_dtypes.bfloat16

D = 2048        # d_model
T = 2048        # seq len per batch
B = 2
NUM_HEADS = 16
NUM_KV = 4
DH = 128        # head dim
HPC = 2         # query heads per core
EC = HPC * DH   # 256 q-channels per core
TS = 512        # token-tile width
NTT = B * T // TS           # 8 global token tiles (4 per batch)
NQ = T // TS                # 4 query tiles per batch
NJ = D // 128               # 16 contraction chunks / s-tiles
N_CORES = 8
GT = B * T                  # 4096 global token rows
SCALE = 1.0 / float(np.sqrt(DH))
RND = 12582912.0            # 1.5 * 2^23: f32 add/sub rounds to nearest int

_CACHE = {}


def _cpack():
    """identity(128) | maskA(1024) | maskB(1024) | ones(1) | ones(128)."""
    ident = np.eye(128, dtype=np.float32)
    tc = np.arange(512)[None, :]
    s = np.arange(128)[:, None]

    def mk(o0, o1):
        return np.concatenate(
            [(tc >= o0 * 128 + s), (tc >= o1 * 128 + s)], axis=1
        ).astype(np.float32)

    ones = np.ones((128, 129), np.float32)
    return np.concatenate([ident, mk(0, 1), mk(2, 3), ones], axis=1).astype(NPBF)


def build_nc():
    if "nc" in _CACHE:
        return _CACHE["nc"]
    nc = bass.Bass(num_devices=N_CORES)
    xs = nc.dram_tensor("xs", [TS, D], BF, kind="ExternalInput").ap()
    Wq = nc.dram_tensor("Wq", [D, EC], BF, kind="ExternalInput").ap()
    Wk = nc.dram_tensor("Wk", [D, DH], BF, kind="ExternalInput").ap()
    Wv = nc.dram_tensor("Wv", [D, DH], BF, kind="ExternalInput").ap()
    Wo = nc.dram_tensor("Wo", [EC, D], BF, kind="ExternalInput").ap()
    bpack_d = nc.dram_tensor("bpack", [128, HPC + 2], F32, kind="ExternalInput").ap()
    y_q = nc.dram_tensor("y_q", [TS, D], I8, kind="ExternalOutput").ap()
    y_s = nc.dram_tensor("y_s", [TS, 1], F32, kind="ExternalOutput").ap()
    cpack_d = nc.inline_tensor(_cpack(), name="cpack")

    grp = [list(range(N_CORES))]

    with tile.TileContext(nc) as tc:
        with (
            tc.tile_pool(name="dram", bufs=1, space="DRAM") as dram,
            tc.tile_pool(name="consts", bufs=1) as consts,
            tc.tile_pool(name="persist", bufs=1) as persist,
            tc.tile_pool(name="wpool", bufs=1) as wpool,
            tc.tile_pool(name="xrows", bufs=1) as xrows,
            tc.tile_pool(name="xapool", bufs=1) as xapool,
            tc.tile_pool(name="expp", bufs=3) as expp,
            tc.tile_pool(name="attp", bufs=8) as attp,
            tc.tile_pool(name="ypool", bufs=4) as ypool,
            tc.tile_pool(name="ytail", bufs=1) as ytail,
            tc.tile_pool(name="small", bufs=4) as small,
        ):
            xs_b = dram.tile([TS, D], BF)
            xg = dram.tile([NTT, TS, D], BF)      # all-gathered bf16 x
            yp = dram.tile([GT, D], F32)          # this core's 2-head partial y
            yrs = dram.tile([TS, D], F32)         # reduce-scattered final rows

            nc.sync.dma_start(out=xs_b, in_=xs)
            nc.gpsimd.collective_compute(
                "AllGather", mybir.AluOpType.bypass, replica_groups=grp,
                ins=[xs_b.opt()], outs=[xg.opt()],
            )

            # ---- constants ------------------------------------------------
            cpack = consts.tile([128, 2305], BF)
            nc.sync.dma_start(out=cpack, in_=cpack_d.ap())
            bpack = consts.tile([128, HPC + 2], F32)
            nc.sync.dma_start(out=bpack, in_=bpack_d)
            identity = cpack[:, 0:128]
            maskA = cpack[:, 128:1152]
            maskB = cpack[:, 1152:2176]
            ones_s = cpack[:, 2176:2177]
            ones_r = cpack[0:1, 2177:2305]
            bq_sb = bpack[:, 0:HPC]
            bk_sb = bpack[:, HPC:HPC + 1]
            bv_sb = bpack[:, HPC + 1:HPC + 2]
            # Pre-touch on DVE: later DVE consumers then carry only one wait.
            pt = consts.tile([128, 16], BF)
            nc.vector.tensor_copy(out=pt, in_=cpack[:, 0:16])
            ptf = consts.tile([128, HPC + 2], F32)
            nc.vector.tensor_copy(out=ptf, in_=bpack)

            # ---- persistent activations -----------------------------------
            QT = [persist.tile([128, T], BF, tag=f"QT{i}", name=f"QT{i}")
                  for i in range(B * HPC)]        # index 2*b + h
            KT = [persist.tile([128, T], BF, tag=f"KT{b}", name=f"KT{b}")
                  for b in range(B)]
            V = [persist.tile([128, NJ, DH], BF, tag=f"V{b}", name=f"V{b}")
                 for b in range(B)]
            Wq_sb = wpool.tile([128, NJ, EC], BF, tag="Wq")
            Wk_sb = wpool.tile([128, NJ, DH], BF, tag="Wk")
            Wv_sb = wpool.tile([128, NJ, DH], BF, tag="Wv")
            Wo_sb = wpool.tile([128, HPC, D], BF, tag="Wo")
            for h in range(HPC):
                nc.sync.dma_start(out=Wo_sb[:, h, :], in_=Wo[h * 128:(h + 1) * 128, :])
            for j in range(NJ):
                nc.sync.dma_start(out=Wq_sb[:, j, :], in_=Wq[j * 128:(j + 1) * 128, :])
                nc.sync.dma_start(out=Wk_sb[:, j, :], in_=Wk[j * 128:(j + 1) * 128, :])
                nc.sync.dma_start(out=Wv_sb[:, j, :], in_=Wv[j * 128:(j + 1) * 128, :])

            # ---- phase A: dequant + transpose x, projections QT/KT/V -------
            with (
                tc.tile_pool(name="psA", bufs=1, space="PSUM") as psA,
                tc.tile_pool(name="psT", bufs=2, space="PSUM") as psT,
            ):
                warm = psT.tile([128, 128], BF, tag="xp")
                nc.tensor.transpose(warm, identity, identity)
                for Tt in range(NTT):
                    b, q = Tt // NQ, Tt % NQ
                    tsl = slice(q * TS, (q + 1) * TS)
                    xr = []
                    for k in range(TS // 128):
                        psl = slice(k * 128, (k + 1) * 128)
                        xt = xrows.tile([128, D], BF, tag=f"xr{k}")
                        nc.sync.dma_start(out=xt, in_=xg[Tt, psl, :])
                        xr.append(xt)
                    xa = []
                    for j in range(NJ):
                        xj = xapool.tile([128, TS], BF, tag=f"xa{j}")
                        for k in range(TS // 128):
                            xp_ps = psT.tile([128, 128], BF, tag="xp")
                            nc.tensor.transpose(
                                xp_ps, xr[k][:, j * 128:(j + 1) * 128], identity)
                            nc.vector.tensor_copy(
                                out=xj[:, k * 128:(k + 1) * 128], in_=xp_ps)
                        xa.append(xj)
                    for h in range(HPC):
                        qt_ps = psA.tile([128, TS], F32, tag=f"qt{h}")
                        for j in range(NJ):
                            nc.tensor.matmul(
                                qt_ps, Wq_sb[:, j, h * 128:(h + 1) * 128], xa[j],
                                start=(j == 0), stop=(j == NJ - 1),
                            )
                        nc.vector.tensor_scalar_add(
                            out=QT[B * b + h][:, tsl], in0=qt_ps,
                            scalar1=bq_sb[:, h:h + 1],
                        )
                    kt_ps = psA.tile([128, TS], F32, tag="kt")
                    for j in range(NJ):
                        nc.tensor.matmul(kt_ps, Wk_sb[:, j, :], xa[j],
                                         start=(j == 0), stop=(j == NJ - 1))
                    nc.vector.tensor_scalar_add(
                        out=KT[b][:, tsl], in0=kt_ps, scalar1=bk_sb,
                    )
                    vt_ps = psA.tile([128, TS], F32, tag="vt")
                    for j in range(NJ):
                        nc.tensor.matmul(vt_ps, Wv_sb[:, j, :], xa[j],
                                         start=(j == 0), stop=(j == NJ - 1))
                    vt_sb = small.tile([128, TS], BF, tag="vt_sb")
                    nc.vector.tensor_scalar_add(
                        out=vt_sb, in0=vt_ps, scalar1=bv_sb,
                    )
                    for k in range(TS // 128):
                        v_ps = psT.tile([128, 128], BF, tag="vp")
                        nc.tensor.transpose(
                            v_ps, vt_sb[:, k * 128:(k + 1) * 128], identity)
                        nc.vector.tensor_copy(out=V[b][:, q * 4 + k, :], in_=v_ps)

            # ---- phase B/C: attention + partial output projection ----------
            with (
                tc.tile_pool(name="psst", bufs=2, space="PSUM") as psst,
                tc.tile_pool(name="psat", bufs=1, space="PSUM") as psat,
                tc.tile_pool(name="psz", bufs=1, space="PSUM") as psz,
                tc.tile_pool(name="psy", bufs=2, space="PSUM") as psy,
            ):
                for b in range(B):
                    for q in range(NQ):
                        tsl = slice(q * TS, (q + 1) * TS)
                        att_sb = []
                        for h in range(HPC):
                            njj = 4 * q + 4       # s-tiles 0 .. 4*q+3
                            ngr = njj // 2
                            at_ps = psat.tile([128, TS], F32, tag="at")
                            z_ps = psz.tile([1, TS], F32, tag="z")
                            for g in range(ngr):
                                j0 = 2 * g
                                st = psst.tile([128, 1024], F32, tag="st")
                                for half in range(2):
                                    j = j0 + half
                                    nc.tensor.matmul(
                                        st[:, half * 512:(half + 1) * 512],
                                        KT[b][:, j * 128:(j + 1) * 128],
                                        QT[B * b + h][:, tsl],
                                        start=True, stop=True,
                                    )
                                ex = expp.tile([128, 1024], BF, tag="ex")
                                nc.scalar.activation(
                                    out=ex, in_=st,
                                    func=mybir.ActivationFunctionType.Exp,
                                    scale=SCALE,
                                )
                                if g == ngr - 2:
                                    nc.vector.tensor_mul(ex, ex, maskA)
                                elif g == ngr - 1:
                                    nc.vector.tensor_mul(ex, ex, maskB)
                                for half in range(2):
                                    j = j0 + half
                                    exh = ex[:, half * 512:(half + 1) * 512]
                                    nc.tensor.matmul(
                                        z_ps, ones_s, exh,
                                        start=(j == 0), stop=(j == njj - 1),
                                    )
                                    nc.tensor.matmul(
                                        at_ps, V[b][:, j, :], exh,
                                        start=(j == 0), stop=(j == njj - 1),
                                    )
                            zr = small.tile([1, TS], F32, tag="zr")
                            nc.vector.reciprocal(out=zr, in_=z_ps)
                            zrb = small.tile([1, TS], BF, tag="zrb")
                            nc.vector.tensor_copy(out=zrb, in_=zr)
                            zb_ps = psz.tile([128, TS], F32, tag="z")
                            nc.tensor.matmul(zb_ps, ones_r, zrb,
                                             start=True, stop=True)
                            zb_sb = small.tile([128, TS], BF, tag="zb_sb")
                            nc.vector.tensor_copy(out=zb_sb, in_=zb_ps)
                            at_sb = attp.tile([128, TS], BF, tag="at_sb")
                            nc.vector.tensor_mul(at_sb, at_ps, zb_sb)
                            att_sb.append(at_sb)
                        # partial output projection for these 512 token rows
                        r0 = (b * NQ + q) * TS
                        for fs in range(4):
                            fsl = slice(fs * 512, (fs + 1) * 512)
                            for tt in range(4):
                                y_ps = psy.tile([128, 512], F32, tag="y")
                                for h in range(HPC):
                                    nc.tensor.matmul(
                                        y_ps,
                                        att_sb[h][:, tt * 128:(tt + 1) * 128],
                                        Wo_sb[:, h, fsl],
                                        start=(h == 0), stop=(h == HPC - 1),
                                    )
                                y_sb = ypool.tile([128, 512], F32, tag="y_sb")
                                nc.vector.tensor_copy(out=y_sb, in_=y_ps)
                                nc.sync.dma_start(
                                    out=yp[r0 + tt * 128: r0 + (tt + 1) * 128, fsl],
                                    in_=y_sb,
                                )

            # ---- reduce partials across all 8 cores; quantize own 512 rows -
            nc.gpsimd.collective_compute(
                "ReduceScatter", mybir.AluOpType.add, replica_groups=grp,
                ins=[yp.opt()], outs=[yrs.opt()],
            )
            for k in range(TS // 128):
                psl = slice(k * 128, (k + 1) * 128)
                yf = ytail.tile([128, D], F32, tag="yf")
                nc.sync.dma_start(out=yf, in_=yrs[psl, :])
                amax = ytail.tile([128, 1], F32, tag="amax")
                nc.vector.tensor_reduce(
                    out=amax, in_=yf, axis=mybir.AxisListType.XYZW,
                    op=mybir.AluOpType.max, apply_absolute_value=True)
                acl = ytail.tile([128, 1], F32, tag="acl")
                nc.vector.tensor_scalar_max(out=acl, in0=amax, scalar1=1e-30)
                qsc = ytail.tile([128, 1], F32, tag="qsc")
                nc.vector.reciprocal(out=qsc, in_=acl)
                qsc2 = ytail.tile([128, 1], F32, tag="qsc2")
                nc.vector.tensor_scalar_mul(out=qsc2, in0=qsc, scalar1=127.0)
                osc = ytail.tile([128, 1], F32, tag="osc")
                nc.vector.tensor_scalar_mul(out=osc, in0=acl, scalar1=1.0 / 127.0)
                nc.sync.dma_start(out=y_s[psl, :], in_=osc)
                t1 = ytail.tile([128, D], F32, tag="t1")
                nc.vector.tensor_scalar(
                    out=t1, in0=yf, scalar1=qsc2, scalar2=RND,
                    op0=mybir.AluOpType.mult, op1=mybir.AluOpType.add)
                t2 = ytail.tile([128, D], F32, tag="t2")
                nc.vector.tensor_scalar_add(out=t2, in0=t1, scalar1=-RND)
                yq = ytail.tile([128, D], I8, tag="yq")
                nc.vector.tensor_copy(out=yq, in_=t2)
                nc.sync.dma_start(out=y_q[psl, :], in_=yq)

    from concourse.bacc import _bass_rust
    _bass_rust.move_matmul_waits_to_ldweights(nc.m)
    _bass_rust.generate_event_semaphores(nc)
    _CACHE["nc"] = nc
    return nc


def _make_runner(nc):
    import jax
    from jax.sharding import Mesh, PartitionSpec, NamedSharding
    from jax.experimental.shard_map import shard_map

    bass2jax.install_neuronx_cc_hook()
    partition_name = nc.partition_id_tensor.name if nc.partition_id_tensor else None
    in_names, out_names, out_avals = [], [], []
    for alloc in nc.m.functions[0].allocations:
        if not isinstance(alloc, mybir.MemoryLocationSet):
            continue
        name = alloc.memorylocations[0].name
        if alloc.kind == "ExternalInput":
            if name != partition_name:
                in_names.append(name)
        elif alloc.kind == "ExternalOutput":
            out_names.append(name)
            out_avals.append(jax.core.ShapedArray(
                tuple(alloc.tensor_shape), mybir.dt.np(alloc.dtype)))
    n_params = len(in_names)
    n_outs = len(out_avals)
    all_names = in_names + out_names
    if partition_name is not None:
        all_names = all_names + [partition_name]
    donate = tuple(range(n_params, n_params + n_outs))

    def _body(*args):
        operands = list(args)
        if partition_name is not None:
            operands.append(bass2jax.partition_id_tensor())
        outs = bass2jax._bass_exec_p.bind(
            *operands, out_avals=tuple(out_avals), in_names=tuple(all_names),
            out_names=tuple(out_names), lowering_input_output_aliases=(),
            sim_require_finite=True, sim_require_nnan=True, nc=nc)
        return tuple(outs)

    devices = jax.devices()[:N_CORES]
    mesh = Mesh(np.asarray(devices), ("core",))
    P = PartitionSpec
    fn = jax.jit(
        shard_map(_body, mesh=mesh, in_specs=(P("core"),) * (n_params + n_outs),
                  out_specs=(P("core"),) * n_outs, check_rep=False),
        donate_argnums=donate, keep_unused=True)

    sh = NamedSharding(mesh, P("core"))
    zshapes = [(N_CORES * av.shape[0], *av.shape[1:]) for av in out_avals]
    zdtypes = [av.dtype for av in out_avals]
    zfn = jax.jit(
        lambda: tuple(jax.numpy.zeros(s, d) for s, d in zip(zshapes, zdtypes)),
        out_shardings=tuple(sh for _ in zshapes))
    return fn, zfn, in_names, out_names, sh


def _fingerprint(arrs):
    fps = []
    for a in arrs:
        v = a.reshape(-1).view(np.uint8)
        fps.append((a.shape, a.dtype.str,
                    int(v.view(np.uint32).sum(dtype=np.uint64))
                    if v.nbytes % 4 == 0 else int(v.sum(dtype=np.uint64)),
                    v[:16].tobytes(), v[-16:].tobytes()))
    return tuple(fps)


def _prep_weights(Wq, bq, Wk, bk, Wv, bv, Wo):
    """Global (concat-over-cores) weight arrays, bf16."""
    Wqb = Wq.astype(NPBF)
    Wkb = Wk.astype(NPBF)
    Wvb = Wv.astype(NPBF)
    Wob = np.ascontiguousarray(Wo).astype(NPBF)
    Wq_g = np.concatenate([Wqb[:, c * EC:(c + 1) * EC] for c in range(N_CORES)], axis=0)
    Wk_g = np.concatenate([Wkb[:, (c // 2) * DH:(c // 2 + 1) * DH]
                           for c in range(N_CORES)], axis=0)
    Wv_g = np.concatenate([Wvb[:, (c // 2) * DH:(c // 2 + 1) * DH]
                           for c in range(N_CORES)], axis=0)
    bp = []
    for c in range(N_CORES):
        g = c // 2
        bp.append(np.concatenate(
            [bq[c * EC:(c + 1) * EC].reshape(HPC, DH).T,
             bk[g * DH:(g + 1) * DH].reshape(DH, 1),
             bv[g * DH:(g + 1) * DH].reshape(DH, 1)], axis=1).astype(np.float32))
    bp_g = np.concatenate(bp, axis=0)
    return {"Wq": Wq_g, "Wk": Wk_g, "Wv": Wv_g, "Wo": Wob, "bpack": bp_g}


def _get_runner():
    if "fn" not in _CACHE:
        nc = build_nc()
        fn, zfn, in_names, out_names, sh = _make_runner(nc)
        _CACHE["fn"] = fn
        _CACHE["zfn"] = zfn
        _CACHE["in_names"] = in_names
        _CACHE["out_names"] = out_names
        _CACHE["sh"] = sh
    return _CACHE["fn"], _CACHE["zfn"], _CACHE["in_names"], _CACHE["sh"]


def _device_weights(Wq, bq, Wk, bk, Wv, bv, Wo, sh):
    import jax
    warrs = [Wq, bq, Wk, bk, Wv, bv, Wo]
    ids = tuple(id(a) for a in warrs)
    if _CACHE.get("w_ids") == ids:
        return _CACHE["w_dev"]
    fp = _fingerprint(warrs)
    if _CACHE.get("w_fp") != fp:
        glob = _prep_weights(Wq, bq, Wk, bk, Wv, bv, Wo)
        _CACHE["w_dev"] = {k: jax.device_put(v, sh) for k, v in glob.items()}
        _CACHE["w_fp"] = fp
    _CACHE["w_ids"] = ids
    _CACHE["w_refs"] = warrs   # hold strong refs so ids stay valid
    return _CACHE["w_dev"]


def kernel(x, Wq, bq, Wk, bk, Wv, bv, Wo, bo):
    x = np.asarray(x)
    Wq = np.asarray(Wq); bq = np.asarray(bq, dtype=np.float32)
    Wk = np.asarray(Wk); bk = np.asarray(bk, dtype=np.float32)
    Wv = np.asarray(Wv); bv = np.asarray(bv, dtype=np.float32)
    Wo = np.asarray(Wo); bo = np.asarray(bo, dtype=np.float32)

    fn, zfn, in_names, sh = _get_runner()
    xs_g = np.ascontiguousarray(x.reshape(GT, D)).astype(NPBF)
    last = None
    for attempt in range(3):
        if attempt > 0:
            # a device reset invalidates cached device buffers; re-upload
            for k in ("w_dev", "w_fp", "w_ids"):
                _CACHE.pop(k, None)
        wdev = _device_weights(Wq, bq, Wk, bk, Wv, bv, Wo, sh)
        args = {"xs": xs_g, **wdev}
        ordered = [args[n] for n in in_names]
        try:
            outs = fn(*ordered, *zfn())
            for o in outs:   # overlap the 8 per-shard D2H copies
                for s in o.addressable_shards:
                    s.data.copy_to_host_async()
            res = {n: np.asarray(o) for n, o in zip(_CACHE["out_names"], outs)}
            break
        except Exception as e:  # transient NRT failures
            last = e
            import time as _t
            _t.sleep(5)
    else:
        raise last
    y = np.multiply(res["y_q"], res["y_s"], dtype=np.float32)
    y += bo
    return y.reshape(B, T, D)
